# revision 3
# baseline (speedup 1.0000x reference)
"""Trainium2 Bass kernel for nn_Conv1dBlock (LIF spikes -> Conv1d(k=5, same) -> GroupNorm).

Contract: kernel(**inputs) takes FULL inputs (x [4,64,256,512] f32, conv_w
[256,256,5], conv_b/gamma/beta [256]) and returns the FULL [4,64,256,512] f32
output. Internally shards data-parallel over B across 8 NeuronCores.

Per-core algorithm (B_loc = 8):
  - LIF (fp32, u = 2*v scaling):
      u = 0.5*m + x (DVE) ; s = (u >= 1) -> fp8 (DVE) ; m = (s==0)*u (GpSimd)
  - Conv1d as fp8 DoubleRow matmuls (2x PE throughput vs bf16): weights
    quantized to e4m3 at scale 2^13; 5 single-precision tap matmuls
    (each contracting all 256 ci via DoubleRow) + 4 residual-correction
    matmuls for taps 0-3 (e4m3 of the quantization residual, same scale,
    accumulated in the same PSUM). Taps 4's residual is dropped: measured
    end-to-end rel err 1.58e-2 vs the 2e-2 gate.
  - GroupNorm stats via DVE bn_stats/bn_aggr per [128,512] PSUM tile
    (per-channel mean/var in one pass; no ScalarE COPY/SQUARE passes).
    Group combine via tiny f32 matmuls (onesg gsum, onesb4 broadcast).
      a = mean + b' ; z = a^2 + var  (b' = 2^13 * conv_b)
      mu_g = sum(a)/32 ; var_g = sum(z)/32 - mu_g^2
      A = gamma / sqrt(var_g + eps') ; B = (b' - mu_g)*A + beta
  - Affine out = A*Y + B on ScalarE (activation Identity with per-channel
    scale/bias APs) reading PSUM directly, writing fp16 SBUF; DMA out fp16
    (host upcasts to f32). Halves output HBM traffic.
"""

import numpy as np
import ml_dtypes

T, B_FULL, C, L, K = 4, 64, 256, 512, 5
N_CORES = 8
B_LOC = B_FULL // N_CORES
G = 8            # groups
GPC = C // G     # 32 channels per group
CT = 2           # 128-channel tiles
EPS = 1e-5
WSCALE = 2.0 ** 13
EPS_S = EPS * WSCALE * WSCALE
NR = 4           # residual-corrected taps (0..3)

_COMPILED = {}


def _build_program():
    import concourse.bass as bass
    import concourse.tile as tile
    from concourse import bacc, mybir

    f32 = mybir.dt.float32
    bf16 = mybir.dt.bfloat16
    fp8 = mybir.dt.float8e4
    f16 = mybir.dt.float16
    Alu = mybir.AluOpType
    Act = mybir.ActivationFunctionType
    DR = mybir.MatmulPerfMode.DoubleRow

    nc = bacc.Bacc(
        "TRN2",
        target_bir_lowering=False,
        debug=False,
        num_devices=N_CORES,
    )

    x_d = nc.dram_tensor("x", [T, B_LOC, C, L], f32, kind="ExternalInput").ap()
    # [ci, k, co_t, ci_t, co] single e4m3 at scale 2^13
    ws_d = nc.dram_tensor("ws", [128, K, CT, 2, 128], fp8, kind="ExternalInput").ap()
    # residuals for taps 0..NR-1, same layout/scale
    wr_d = nc.dram_tensor("wr", [128, NR, CT, 2, 128], fp8, kind="ExternalInput").ap()
    # [co, field, m(rep), co_t]; fields: b' (=2^13 b), gamma, beta
    chan_d = nc.dram_tensor("chan", [128, 3, 2, CT], f32, kind="ExternalInput").ap()
    onesg_d = nc.dram_tensor("onesg", [128, 4], f32, kind="ExternalInput").ap()
    onesb4_d = nc.dram_tensor("onesb4", [4, 128], f32, kind="ExternalInput").ap()
    y_d = nc.dram_tensor("y", [T, B_LOC, C, L], f16, kind="ExternalOutput").ap()

    with tile.TileContext(nc) as tc:
        with (
            tc.tile_pool(name="singles", bufs=1) as singles,
            tc.tile_pool(name="xp", bufs=10) as xp,
            tc.tile_pool(name="sp", bufs=6) as sp,
            tc.tile_pool(name="ysb", bufs=8) as ysb,
            tc.tile_pool(name="smallsb", bufs=10) as smallsb,
            tc.tile_pool(name="ypsum", bufs=6, space="PSUM") as ypsum,
            tc.tile_pool(name="spsum", bufs=2, space="PSUM") as spsum,
        ):
            # PE p-state warmup: dummy matmuls on a memset tile keep PE busy
            # from ~0 so real convs start at full clock (off critical path --
            # they overlap the initial x DMA + LIF).
            warm_sb = singles.tile([128, 64], bf16)
            nc.vector.memset(warm_sb[:], 0.25)
            warm_ps = spsum.tile([128, 32], f32, name="small_ps")
            for _ in range(170):
                nc.tensor.matmul(
                    warm_ps[0:64, 0:8], warm_sb[:, 0:64], warm_sb[:, 0:8],
                    start=True, stop=True, skip_group_check=True,
                )
            first_small_ps = warm_ps
            # first input tiles first (LIF -> conv chain gates startup)
            early_x = {}
            for b in range(2):
                xt = xp.tile([128, 2, L], f32)
                nc.sync.dma_start(
                    out=xt[:], in_=x_d[0, b].rearrange("(i p) l -> p i l", p=128)
                )
                early_x[(0, b)] = xt
            ws = singles.tile([128, K, CT, 2, 128], fp8)
            nc.sync.dma_start(out=ws[:], in_=ws_d[:])
            wr = singles.tile([128, NR, CT, 2, 128], fp8)
            nc.sync.dma_start(out=wr[:], in_=wr_d[:])
            onesg = singles.tile([128, 4], f32)
            nc.sync.dma_start(out=onesg[:], in_=onesg_d[:])
            onesb4 = singles.tile([4, 128], f32)
            nc.sync.dma_start(out=onesb4[:], in_=onesb4_d[:])
            chan = singles.tile([128, 3, 2, CT], f32)
            nc.sync.dma_start(out=chan[:], in_=chan_d[:])
            eps_t = singles.tile([128, 1], f32)
            nc.vector.memset(eps_t[:], EPS_S)
            # pre-load the activation table (Sqrt selects sqrt_and_others,
            # which also holds Copy/Identity/Square) off the critical path
            eps_s = singles.tile([128, 1], f32)
            nc.scalar.activation(out=eps_s[0:1], in_=eps_t[0:1], func=Act.Sqrt)

            # persistent LIF membrane state (u = 2v scaling) per local batch;
            # first written at t=0 (no memset needed)
            m_tiles = []
            for b in range(B_LOC):
                mt = singles.tile([128, 2, L], f32, tag=f"m{b}")
                m_tiles.append(mt)

            # tap -> (rhs_lo, rhs_hi, out_lo, out_hi) column ranges
            tap_slices = []
            for k in range(K):
                d = k - 2
                if d >= 0:
                    tap_slices.append((d, L, 0, L - d))
                else:
                    tap_slices.append((0, L + d, -d, L))

            # matmul order: tap 2 single first (full-width start=True zeroes
            # the whole bank), then remaining singles, then residuals
            mm_list = [("s", 2), ("s", 0), ("s", 1), ("s", 3), ("s", 4)]
            mm_list += [("r", j) for j in range(NR)]
            n_mm = len(mm_list)

            def tail_front(pend):
                """Pair-tail stage 1: a/z per-channel stats (DVE) and the
                group-sum f32 matmul (PE)."""
                tb_pair, small_ps, stats, az, yps = pend
                gm = stats.shape[1]
                # a = mean + b'
                nc.vector.tensor_add(
                    out=az[:, :, :, 0], in0=stats[:, :, :, 0], in1=chan[:, 0, 0:gm]
                )
                # z = a^2 + var
                tmp = smallsb.tile([128, gm, CT], f32)
                nc.vector.tensor_mul(out=tmp[:], in0=az[:, :, :, 0], in1=az[:, :, :, 0])
                nc.vector.tensor_add(out=az[:, :, :, 1], in0=tmp[:], in1=stats[:, :, :, 1])
                # group sums: [4, gm*CT*2] f32 matmul
                nc.tensor.matmul(
                    small_ps[0:4, 0 : gm * 4], onesg[:], az[:],
                    start=True, stop=True,
                )

            def tail_mid(pend):
                """Pair-tail stage 2: mu/kappa chain (DVE+Act), broadcast
                matmul (PE), A/B coefficients (DVE)."""
                tb_pair, small_ps, stats, az, yps = pend
                gm = stats.shape[1]
                gsum = small_ps[0:4, 0 : gm * 4].rearrange(
                    "p (m c s) -> p m c s", m=gm, c=CT
                )
                mk = smallsb.tile([4, gm, CT, 2], f32)  # (mu, kappa)
                m2 = smallsb.tile([4, gm, CT], f32)
                vr = smallsb.tile([4, gm, CT], f32)
                mu_v = mk[0:4, :, :, 0]
                nc.vector.tensor_scalar(
                    out=mu_v, in0=gsum[:, :, :, 0], scalar1=1.0 / GPC,
                    scalar2=None, op0=Alu.mult,
                )
                nc.vector.tensor_mul(out=m2[:], in0=mu_v, in1=mu_v)
                nc.vector.scalar_tensor_tensor(
                    out=vr[:], in0=gsum[:, :, :, 1], scalar=1.0 / GPC, in1=m2[:],
                    op0=Alu.mult, op1=Alu.subtract,
                )
                nc.scalar.activation(
                    out=vr[:], in_=vr[:], func=Act.Sqrt, bias=eps_t[0:4],
                )
                nc.vector.reciprocal(out=mk[0:4, :, :, 1], in_=vr[:])
                # broadcast groups -> channels: [128, gm*CT*2] f32 matmul
                nc.tensor.matmul(
                    small_ps[:, 16 : 16 + gm * 4], onesb4[:], mk[:],
                    start=True, stop=True,
                )
                bcv = small_ps[:, 16 : 16 + gm * 4].rearrange(
                    "p (m c s) -> p m c s", m=gm, c=CT
                )
                # A = kappa * gamma ; B = (b' - mu) * A + beta
                ab = smallsb.tile([128, gm, CT, 2], f32)
                tmp = smallsb.tile([128, gm, CT], f32)
                nc.vector.tensor_mul(
                    out=ab[:, :, :, 0], in0=bcv[:, :, :, 1], in1=chan[:, 1, 0:gm]
                )
                nc.vector.tensor_sub(
                    out=tmp[:], in0=chan[:, 0, 0:gm], in1=bcv[:, :, :, 0]
                )
                nc.vector.tensor_mul(out=tmp[:], in0=tmp[:], in1=ab[:, :, :, 0])
                nc.vector.tensor_add(
                    out=ab[:, :, :, 1], in0=tmp[:], in1=chan[:, 2, 0:gm]
                )
                return ab

            def tail_store(pend, ab):
                """Pair-tail stage 3: out = A*Y + B on ScalarE (PSUM -> fp16
                SBUF) and DMA out."""
                tb_pair, small_ps, stats, az, yps = pend
                for mi in range(len(tb_pair)):
                    t, b = tb_pair[mi]
                    for ct in range(CT):
                        y_sb = ysb.tile([128, L], f16)
                        nc.scalar.activation(
                            out=y_sb[:], in_=yps[mi][ct][:], func=Act.Identity,
                            scale=ab[:, mi, ct, 0:1], bias=ab[:, mi, ct, 1:2],
                        )
                        nc.sync.dma_start(
                            out=y_d[t, b].rearrange("(i p) l -> p i l", p=128)[:, ct, :],
                            in_=y_sb[:],
                        )

            groups = [(2 * i, 2 * i + 1) for i in range(16)]
            gof = {}
            for g in groups:
                for j, s_ in enumerate(g):
                    gof[s_] = (g, j)
            pending = None
            cur = None
            for t in range(T):
                for b in range(B_LOC):
                    idx = t * B_LOC + b
                    grp, mi = gof[idx]
                    gsz = len(grp)
                    if mi == 0:
                        if pending is not None:
                            tail_front(pending)
                        if idx == 0:
                            small_ps = first_small_ps
                        else:
                            small_ps = spsum.tile([128, 32], f32, name="small_ps")
                        stats = smallsb.tile([128, gsz, CT, 2], f32, name="stats")
                        az = smallsb.tile([128, gsz, CT, 2], f32, name="az")
                        cur = ([None] * gsz, small_ps, stats, az, [None] * gsz)
                    cur[0][mi] = (t, b)

                    xt = early_x.pop((t, b), None)
                    if xt is None:
                        xt = xp.tile([128, 2, L], f32)
                        nc.sync.dma_start(
                            out=xt[:],
                            in_=x_d[t, b].rearrange("(i p) l -> p i l", p=128),
                        )
                    mt = m_tiles[b]
                    st = sp.tile([128, 2, L], fp8)
                    if t == 0:
                        # m uninitialized: u = x exactly; spike on Pool
                        nc.gpsimd.tensor_scalar(
                            out=st[:], in0=xt[:], scalar1=1.0, scalar2=None,
                            op0=Alu.is_ge,
                        )
                        # m = (s==0)*x
                        nc.vector.scalar_tensor_tensor(
                            out=mt[:], in0=st[:], scalar=0.0, in1=xt[:],
                            op0=Alu.is_equal, op1=Alu.mult,
                        )
                    else:
                        # LIF step: u = 0.5*m + x ; s = (u>=1) ; m = (s==0)*u
                        nc.vector.scalar_tensor_tensor(
                            out=mt[:], in0=mt[:], scalar=0.5, in1=xt[:],
                            op0=Alu.mult, op1=Alu.add,
                        )
                        nc.gpsimd.tensor_scalar(
                            out=st[:], in0=mt[:], scalar1=1.0, scalar2=None,
                            op0=Alu.is_ge,
                        )
                        if t < T - 1:
                            nc.vector.scalar_tensor_tensor(
                                out=mt[:], in0=st[:], scalar=0.0, in1=mt[:],
                                op0=Alu.is_equal, op1=Alu.mult,
                            )

                    # conv + stats per co-tile
                    yps = []
                    for ct in range(CT):
                        yp = ypsum.tile([128, L], f32)
                        for i, (kind, k) in enumerate(mm_list):
                            rl, rh, ol, oh = tap_slices[k]
                            w_ap = ws[:, k, ct] if kind == "s" else wr[:, k, ct]
                            nc.tensor.matmul(
                                yp[:, ol:oh],
                                w_ap,
                                st[:, :, rl:rh],
                                start=(i == 0),
                                stop=(i == n_mm - 1),
                                perf_mode=DR,
                                skip_group_check=True,
                            )
                        bns = smallsb.tile([128, 6], f32)
                        nc.vector.bn_stats(out=bns[:], in_=yp[:])
                        nc.vector.bn_aggr(out=cur[2][:, mi, ct, :], in_=bns[:])
                        yps.append(yp)
                    cur[4][mi] = yps

                    if mi == 0 and pending is not None:
                        ab = tail_mid(pending)
                        tail_store(pending, ab)
                        pending = None
                    if mi == gsz - 1:
                        pending = cur
            tail_front(pending)
            ab = tail_mid(pending)
            tail_store(pending, ab)

    nc.compile()
    return nc


def _prep_host_inputs(x, conv_w, conv_b, gamma, beta):
    x = np.asarray(x, dtype=np.float32)
    conv_w = np.asarray(conv_w, dtype=np.float32)
    conv_b = np.asarray(conv_b, dtype=np.float32)
    gamma = np.asarray(gamma, dtype=np.float32)
    beta = np.asarray(beta, dtype=np.float32)

    def q8(a):
        return a.astype(ml_dtypes.float8_e4m3).astype(np.float32)

    # [ci_t, ci, co_t, co, k] at scale 2^13
    Wt = conv_w.transpose(1, 0, 2)                      # [ci_g, co_g, k]
    W6 = Wt.reshape(2, 128, CT, 128, K) * np.float32(WSCALE)
    w8 = q8(W6)
    r8 = q8(W6 - w8)
    # ws[ci, k, ct, ci_t, co]
    ws_host = np.ascontiguousarray(
        w8.transpose(1, 4, 2, 0, 3).astype(ml_dtypes.float8_e4m3)
    )
    # wr[ci, j(tap), ct, ci_t, co] for taps 0..NR-1
    wr_host = np.ascontiguousarray(
        r8[:, :, :, :, 0:NR].transpose(1, 4, 2, 0, 3).astype(ml_dtypes.float8_e4m3)
    )

    fields = np.stack([conv_b * np.float32(WSCALE), gamma, beta])  # [3, 256]
    chan1 = fields.reshape(3, CT, 128).transpose(2, 0, 1)          # [128, 3, ct]
    chan = np.ascontiguousarray(
        np.broadcast_to(chan1[:, :, None, :], (128, 3, 2, CT))
    )

    onesg = np.zeros((128, 4), np.float32)
    for ci in range(128):
        onesg[ci, ci // GPC] = 1.0
    onesb4 = np.zeros((4, 128), np.float32)
    for co in range(128):
        onesb4[co // GPC, co] = 1.0

    shards = []
    for i in range(N_CORES):
        shards.append(
            {
                "x": np.ascontiguousarray(x[:, i * B_LOC : (i + 1) * B_LOC]),
                "ws": ws_host,
                "wr": wr_host,
                "chan": chan,
                "onesg": onesg,
                "onesb4": onesb4,
            }
        )
    return shards


def kernel(x, conv_w, conv_b, gamma, beta, _trace=False):
    from concourse.bass_utils import run_bass_kernel_spmd

    if "nc" not in _COMPILED:
        _COMPILED["nc"] = _build_program()
    nc = _COMPILED["nc"]

    in_maps = _prep_host_inputs(x, conv_w, conv_b, gamma, beta)
    res = run_bass_kernel_spmd(
        nc, in_maps, list(range(N_CORES)), trace=_trace
    )
    out = np.concatenate([r["y"] for r in res.results], axis=1).astype(np.float32)
    _COMPILED["last_result"] = res
    return out


# revision 4
# speedup vs baseline: 2.4385x; 2.4385x over previous
"""Trainium2 Bass kernel for nn_Conv1dBlock (LIF spikes -> Conv1d(k=5, same) -> GroupNorm).

Contract: kernel(**inputs) takes FULL inputs (x [4,64,256,512] f32, conv_w
[256,256,5], conv_b/gamma/beta [256]) and returns the FULL [4,64,256,512] f32
output. Internally shards data-parallel over B across 8 NeuronCores.

Per-core algorithm (B_loc = 8):
  - LIF (fp32, u = 2*v scaling):
      u = 0.5*m + x (DVE) ; s = (u >= 1) -> fp8 (DVE) ; m = (s==0)*u (GpSimd)
  - Conv1d as fp8 DoubleRow matmuls (2x PE throughput vs bf16): weights
    quantized to e4m3 at scale 2^13; 5 single-precision tap matmuls
    (each contracting all 256 ci via DoubleRow) + 4 residual-correction
    matmuls for taps 0-3 (e4m3 of the quantization residual, same scale,
    accumulated in the same PSUM). Taps 4's residual is dropped: measured
    end-to-end rel err 1.58e-2 vs the 2e-2 gate.
  - GroupNorm stats via DVE bn_stats/bn_aggr per [128,512] PSUM tile
    (per-channel mean/var in one pass; no ScalarE COPY/SQUARE passes).
    Group combine via tiny f32 matmuls (onesg gsum, onesb4 broadcast).
      a = mean + b' ; z = a^2 + var  (b' = 2^13 * conv_b)
      mu_g = sum(a)/32 ; var_g = sum(z)/32 - mu_g^2
      A = gamma / sqrt(var_g + eps') ; B = (b' - mu_g)*A + beta
  - Affine out = A*Y + B on ScalarE (activation Identity with per-channel
    scale/bias APs) reading PSUM directly, writing fp16 SBUF; DMA out fp16
    (host upcasts to f32). Halves output HBM traffic.
"""

import numpy as np
import ml_dtypes

T, B_FULL, C, L, K = 4, 64, 256, 512, 5
N_CORES = 8
B_LOC = B_FULL // N_CORES
G = 8            # groups
GPC = C // G     # 32 channels per group
CT = 2           # 128-channel tiles
EPS = 1e-5
WSCALE = 2.0 ** 13
EPS_S = EPS * WSCALE * WSCALE
NR = 4           # residual-corrected taps (0..3)

_COMPILED = {}


def _build_program():
    import concourse.bass as bass
    import concourse.tile as tile
    from concourse import bacc, mybir

    f32 = mybir.dt.float32
    bf16 = mybir.dt.bfloat16
    fp8 = mybir.dt.float8e4
    f16 = mybir.dt.float16
    Alu = mybir.AluOpType
    Act = mybir.ActivationFunctionType
    DR = mybir.MatmulPerfMode.DoubleRow

    nc = bacc.Bacc(
        "TRN2",
        target_bir_lowering=False,
        debug=False,
        num_devices=N_CORES,
    )

    x_d = nc.dram_tensor("x", [T, B_LOC, C, L], f32, kind="ExternalInput").ap()
    # [ci, k, co_t, ci_t, co] single e4m3 at scale 2^13
    ws_d = nc.dram_tensor("ws", [128, K, CT, 2, 128], fp8, kind="ExternalInput").ap()
    # residuals for taps 0..NR-1, same layout/scale
    wr_d = nc.dram_tensor("wr", [128, NR, CT, 2, 128], fp8, kind="ExternalInput").ap()
    # [co, field, m(rep), co_t]; fields: b' (=2^13 b), gamma, beta
    chan_d = nc.dram_tensor("chan", [128, 3, 2, CT], f32, kind="ExternalInput").ap()
    onesg_d = nc.dram_tensor("onesg", [128, 4], f32, kind="ExternalInput").ap()
    onesb4_d = nc.dram_tensor("onesb4", [4, 128], f32, kind="ExternalInput").ap()
    y_d = nc.dram_tensor("y", [T, B_LOC, C, L], f16, kind="ExternalOutput").ap()

    with tile.TileContext(nc) as tc:
        with (
            tc.tile_pool(name="singles", bufs=1) as singles,
            tc.tile_pool(name="xp", bufs=10) as xp,
            tc.tile_pool(name="sp", bufs=6) as sp,
            tc.tile_pool(name="ysb", bufs=8) as ysb,
            tc.tile_pool(name="smallsb", bufs=10) as smallsb,
            tc.tile_pool(name="ypsum", bufs=6, space="PSUM") as ypsum,
            tc.tile_pool(name="spsum", bufs=2, space="PSUM") as spsum,
        ):
            # PE p-state warmup: dummy matmuls on a memset tile keep PE busy
            # from ~0 so real convs start at full clock (off critical path --
            # they overlap the initial x DMA + LIF).
            warm_sb = singles.tile([128, 64], bf16)
            nc.vector.memset(warm_sb[:], 0.25)
            warm_ps = spsum.tile([128, 32], f32, name="small_ps")
            for _ in range(170):
                nc.tensor.matmul(
                    warm_ps[0:64, 0:8], warm_sb[:, 0:64], warm_sb[:, 0:8],
                    start=True, stop=True, skip_group_check=True,
                )
            first_small_ps = warm_ps
            # first input tiles first (LIF -> conv chain gates startup)
            early_x = {}
            for b in range(2):
                xt = xp.tile([128, 2, L], f32)
                nc.sync.dma_start(
                    out=xt[:], in_=x_d[0, b].rearrange("(i p) l -> p i l", p=128)
                )
                early_x[(0, b)] = xt
            ws = singles.tile([128, K, CT, 2, 128], fp8)
            nc.sync.dma_start(out=ws[:], in_=ws_d[:])
            wr = singles.tile([128, NR, CT, 2, 128], fp8)
            nc.sync.dma_start(out=wr[:], in_=wr_d[:])
            onesg = singles.tile([128, 4], f32)
            nc.sync.dma_start(out=onesg[:], in_=onesg_d[:])
            onesb4 = singles.tile([4, 128], f32)
            nc.sync.dma_start(out=onesb4[:], in_=onesb4_d[:])
            chan = singles.tile([128, 3, 2, CT], f32)
            nc.sync.dma_start(out=chan[:], in_=chan_d[:])
            eps_t = singles.tile([128, 1], f32)
            nc.vector.memset(eps_t[:], EPS_S)
            # pre-load the activation table (Sqrt selects sqrt_and_others,
            # which also holds Copy/Identity/Square) off the critical path
            eps_s = singles.tile([128, 1], f32)
            nc.scalar.activation(out=eps_s[0:1], in_=eps_t[0:1], func=Act.Sqrt)

            # persistent LIF membrane state (u = 2v scaling) per local batch;
            # first written at t=0 (no memset needed)
            m_tiles = []
            for b in range(B_LOC):
                mt = singles.tile([128, 2, L], f32, tag=f"m{b}")
                m_tiles.append(mt)

            # tap -> (rhs_lo, rhs_hi, out_lo, out_hi) column ranges
            tap_slices = []
            for k in range(K):
                d = k - 2
                if d >= 0:
                    tap_slices.append((d, L, 0, L - d))
                else:
                    tap_slices.append((0, L + d, -d, L))

            # matmul order: tap 2 single first (full-width start=True zeroes
            # the whole bank), then remaining singles, then residuals
            mm_list = [("s", 2), ("s", 0), ("s", 1), ("s", 3), ("s", 4)]
            mm_list += [("r", j) for j in range(NR)]
            n_mm = len(mm_list)

            def tail_front(pend):
                """Pair-tail stage 1: a/z per-channel stats (DVE) and the
                group-sum f32 matmul (PE)."""
                tb_pair, small_ps, stats, az, yps = pend
                gm = stats.shape[1]
                # a = mean + b'
                nc.vector.tensor_add(
                    out=az[:, :, :, 0], in0=stats[:, :, :, 0], in1=chan[:, 0, 0:gm]
                )
                # z = a^2 + var
                tmp = smallsb.tile([128, gm, CT], f32)
                nc.vector.tensor_mul(out=tmp[:], in0=az[:, :, :, 0], in1=az[:, :, :, 0])
                nc.vector.tensor_add(out=az[:, :, :, 1], in0=tmp[:], in1=stats[:, :, :, 1])
                # group sums: [4, gm*CT*2] f32 matmul
                nc.tensor.matmul(
                    small_ps[0:4, 0 : gm * 4], onesg[:], az[:],
                    start=True, stop=True,
                )

            def tail_mid(pend):
                """Pair-tail stage 2: mu/kappa chain (DVE+Act), broadcast
                matmul (PE), A/B coefficients (DVE)."""
                tb_pair, small_ps, stats, az, yps = pend
                gm = stats.shape[1]
                gsum = small_ps[0:4, 0 : gm * 4].rearrange(
                    "p (m c s) -> p m c s", m=gm, c=CT
                )
                mk = smallsb.tile([4, gm, CT, 2], f32)  # (mu, kappa)
                m2 = smallsb.tile([4, gm, CT], f32)
                vr = smallsb.tile([4, gm, CT], f32)
                mu_v = mk[0:4, :, :, 0]
                nc.vector.tensor_scalar(
                    out=mu_v, in0=gsum[:, :, :, 0], scalar1=1.0 / GPC,
                    scalar2=None, op0=Alu.mult,
                )
                nc.vector.tensor_mul(out=m2[:], in0=mu_v, in1=mu_v)
                nc.vector.scalar_tensor_tensor(
                    out=vr[:], in0=gsum[:, :, :, 1], scalar=1.0 / GPC, in1=m2[:],
                    op0=Alu.mult, op1=Alu.subtract,
                )
                nc.scalar.activation(
                    out=vr[:], in_=vr[:], func=Act.Sqrt, bias=eps_t[0:4],
                )
                nc.vector.reciprocal(out=mk[0:4, :, :, 1], in_=vr[:])
                # broadcast groups -> channels: [128, gm*CT*2] f32 matmul
                nc.tensor.matmul(
                    small_ps[:, 16 : 16 + gm * 4], onesb4[:], mk[:],
                    start=True, stop=True,
                )
                bcv = small_ps[:, 16 : 16 + gm * 4].rearrange(
                    "p (m c s) -> p m c s", m=gm, c=CT
                )
                # A = kappa * gamma ; B = (b' - mu) * A + beta
                ab = smallsb.tile([128, gm, CT, 2], f32)
                tmp = smallsb.tile([128, gm, CT], f32)
                nc.vector.tensor_mul(
                    out=ab[:, :, :, 0], in0=bcv[:, :, :, 1], in1=chan[:, 1, 0:gm]
                )
                nc.vector.tensor_sub(
                    out=tmp[:], in0=chan[:, 0, 0:gm], in1=bcv[:, :, :, 0]
                )
                nc.vector.tensor_mul(out=tmp[:], in0=tmp[:], in1=ab[:, :, :, 0])
                nc.vector.tensor_add(
                    out=ab[:, :, :, 1], in0=tmp[:], in1=chan[:, 2, 0:gm]
                )
                return ab

            def tail_store(pend, ab):
                """Pair-tail stage 3: out = A*Y + B on ScalarE (PSUM -> fp16
                SBUF) and DMA out."""
                tb_pair, small_ps, stats, az, yps = pend
                for mi in range(len(tb_pair)):
                    t, b = tb_pair[mi]
                    for ct in range(CT):
                        y_sb = ysb.tile([128, L], f16)
                        nc.scalar.activation(
                            out=y_sb[:], in_=yps[mi][ct][:], func=Act.Identity,
                            scale=ab[:, mi, ct, 0:1], bias=ab[:, mi, ct, 1:2],
                        )
                        nc.sync.dma_start(
                            out=y_d[t, b].rearrange("(i p) l -> p i l", p=128)[:, ct, :],
                            in_=y_sb[:],
                        )

            groups = [(2 * i, 2 * i + 1) for i in range(16)]
            gof = {}
            for g in groups:
                for j, s_ in enumerate(g):
                    gof[s_] = (g, j)
            pending = None
            cur = None
            for t in range(T):
                for b in range(B_LOC):
                    idx = t * B_LOC + b
                    grp, mi = gof[idx]
                    gsz = len(grp)
                    if mi == 0:
                        if pending is not None:
                            tail_front(pending)
                        if idx == 0:
                            small_ps = first_small_ps
                        else:
                            small_ps = spsum.tile([128, 32], f32, name="small_ps")
                        stats = smallsb.tile([128, gsz, CT, 2], f32, name="stats")
                        az = smallsb.tile([128, gsz, CT, 2], f32, name="az")
                        cur = ([None] * gsz, small_ps, stats, az, [None] * gsz)
                    cur[0][mi] = (t, b)

                    xt = early_x.pop((t, b), None)
                    if xt is None:
                        xt = xp.tile([128, 2, L], f32)
                        nc.sync.dma_start(
                            out=xt[:],
                            in_=x_d[t, b].rearrange("(i p) l -> p i l", p=128),
                        )
                    mt = m_tiles[b]
                    st = sp.tile([128, 2, L], fp8)

                    def spike(src):
                        # A/B test: even b writes fp8 straight from DVE; odd b
                        # writes bf16 on DVE then casts to fp8 on ScalarE
                        if b % 2 == 0:
                            nc.vector.tensor_scalar(
                                out=st[:], in0=src[:], scalar1=1.0, scalar2=None,
                                op0=Alu.is_ge,
                            )
                        else:
                            sb16 = sp.tile([128, 2, L], bf16, name="sb16")
                            nc.vector.tensor_scalar(
                                out=sb16[:], in0=src[:], scalar1=1.0, scalar2=None,
                                op0=Alu.is_ge,
                            )
                            nc.scalar.activation(out=st[:], in_=sb16[:], func=Act.Copy)

                    if t == 0:
                        # m uninitialized: u = x exactly
                        spike(xt)
                        nc.vector.scalar_tensor_tensor(
                            out=mt[:], in0=xt[:], scalar=1.0, in1=xt[:],
                            op0=Alu.is_lt, op1=Alu.mult,
                        )
                    else:
                        # LIF step: u = 0.5*m + x ; s = (u>=1) ; m = (u<1)*u
                        nc.vector.scalar_tensor_tensor(
                            out=mt[:], in0=mt[:], scalar=0.5, in1=xt[:],
                            op0=Alu.mult, op1=Alu.add,
                        )
                        spike(mt)
                        if t < T - 1:
                            nc.vector.scalar_tensor_tensor(
                                out=mt[:], in0=mt[:], scalar=1.0, in1=mt[:],
                                op0=Alu.is_lt, op1=Alu.mult,
                            )

                    # conv + stats per co-tile
                    yps = []
                    for ct in range(CT):
                        yp = ypsum.tile([128, L], f32)
                        for i, (kind, k) in enumerate(mm_list):
                            rl, rh, ol, oh = tap_slices[k]
                            w_ap = ws[:, k, ct] if kind == "s" else wr[:, k, ct]
                            nc.tensor.matmul(
                                yp[:, ol:oh],
                                w_ap,
                                st[:, :, rl:rh],
                                start=(i == 0),
                                stop=(i == n_mm - 1),
                                perf_mode=DR,
                                skip_group_check=True,
                            )
                        bns = smallsb.tile([128, 6], f32)
                        nc.vector.bn_stats(out=bns[:], in_=yp[:])
                        nc.vector.bn_aggr(out=cur[2][:, mi, ct, :], in_=bns[:])
                        yps.append(yp)
                    cur[4][mi] = yps

                    if mi == 0 and pending is not None:
                        ab = tail_mid(pending)
                        tail_store(pending, ab)
                        pending = None
                    if mi == gsz - 1:
                        pending = cur
            tail_front(pending)
            ab = tail_mid(pending)
            tail_store(pending, ab)

    nc.compile()
    return nc


def _prep_host_inputs(x, conv_w, conv_b, gamma, beta):
    x = np.asarray(x, dtype=np.float32)
    conv_w = np.asarray(conv_w, dtype=np.float32)
    conv_b = np.asarray(conv_b, dtype=np.float32)
    gamma = np.asarray(gamma, dtype=np.float32)
    beta = np.asarray(beta, dtype=np.float32)

    def q8(a):
        return a.astype(ml_dtypes.float8_e4m3).astype(np.float32)

    # [ci_t, ci, co_t, co, k] at scale 2^13
    Wt = conv_w.transpose(1, 0, 2)                      # [ci_g, co_g, k]
    W6 = Wt.reshape(2, 128, CT, 128, K) * np.float32(WSCALE)
    w8 = q8(W6)
    r8 = q8(W6 - w8)
    # ws[ci, k, ct, ci_t, co]
    ws_host = np.ascontiguousarray(
        w8.transpose(1, 4, 2, 0, 3).astype(ml_dtypes.float8_e4m3)
    )
    # wr[ci, j(tap), ct, ci_t, co] for taps 0..NR-1
    wr_host = np.ascontiguousarray(
        r8[:, :, :, :, 0:NR].transpose(1, 4, 2, 0, 3).astype(ml_dtypes.float8_e4m3)
    )

    fields = np.stack([conv_b * np.float32(WSCALE), gamma, beta])  # [3, 256]
    chan1 = fields.reshape(3, CT, 128).transpose(2, 0, 1)          # [128, 3, ct]
    chan = np.ascontiguousarray(
        np.broadcast_to(chan1[:, :, None, :], (128, 3, 2, CT))
    )

    onesg = np.zeros((128, 4), np.float32)
    for ci in range(128):
        onesg[ci, ci // GPC] = 1.0
    onesb4 = np.zeros((4, 128), np.float32)
    for co in range(128):
        onesb4[co // GPC, co] = 1.0

    shards = []
    for i in range(N_CORES):
        shards.append(
            {
                "x": np.ascontiguousarray(x[:, i * B_LOC : (i + 1) * B_LOC]),
                "ws": ws_host,
                "wr": wr_host,
                "chan": chan,
                "onesg": onesg,
                "onesb4": onesb4,
            }
        )
    return shards


def kernel(x, conv_w, conv_b, gamma, beta, _trace=False):
    from concourse.bass_utils import run_bass_kernel_spmd

    if "nc" not in _COMPILED:
        _COMPILED["nc"] = _build_program()
    nc = _COMPILED["nc"]

    in_maps = _prep_host_inputs(x, conv_w, conv_b, gamma, beta)
    res = run_bass_kernel_spmd(
        nc, in_maps, list(range(N_CORES)), trace=_trace
    )
    out = np.concatenate([r["y"] for r in res.results], axis=1).astype(np.float32)
    _COMPILED["last_result"] = res
    return out


# revision 10
# speedup vs baseline: 2.8159x; 1.1548x over previous
"""Trainium2 Bass kernel for nn_Conv1dBlock (LIF spikes -> Conv1d(k=5, same) -> GroupNorm).

Contract: kernel(**inputs) takes FULL inputs (x [4,64,256,512] f32, conv_w
[256,256,5], conv_b/gamma/beta [256]) and returns the FULL [4,64,256,512] f32
output. Internally shards data-parallel over B across 8 NeuronCores.

Per-core algorithm (B_loc = 8):
  - LIF (fp32, u = 2*v scaling):
      u = 0.5*m + x (DVE) ; s = (u >= 1) -> fp8 (DVE) ; m = (s==0)*u (GpSimd)
  - Conv1d as fp8 DoubleRow matmuls (2x PE throughput vs bf16): weights
    quantized to e4m3 at scale 2^13; 5 single-precision tap matmuls
    (each contracting all 256 ci via DoubleRow) + 4 residual-correction
    matmuls for taps 0-3 (e4m3 of the quantization residual, same scale,
    accumulated in the same PSUM). Taps 4's residual is dropped: measured
    end-to-end rel err 1.58e-2 vs the 2e-2 gate.
  - GroupNorm stats via DVE bn_stats/bn_aggr per [128,512] PSUM tile
    (per-channel mean/var in one pass; no ScalarE COPY/SQUARE passes).
    Group combine via tiny f32 matmuls (onesg gsum, onesb4 broadcast).
      a = mean + b' ; z = a^2 + var  (b' = 2^13 * conv_b)
      mu_g = sum(a)/32 ; var_g = sum(z)/32 - mu_g^2
      A = gamma / sqrt(var_g + eps') ; B = (b' - mu_g)*A + beta
  - Affine out = A*Y + B on ScalarE (activation Identity with per-channel
    scale/bias APs) reading PSUM directly, writing fp16 SBUF; DMA out fp16
    (host upcasts to f32). Halves output HBM traffic.
"""

import numpy as np
import ml_dtypes

T, B_FULL, C, L, K = 4, 64, 256, 512, 5
N_CORES = 8
B_LOC = B_FULL // N_CORES
G = 8            # groups
GPC = C // G     # 32 channels per group
CT = 2           # 128-channel tiles
EPS = 1e-5
WSCALE = 2.0 ** 13
EPS_S = EPS * WSCALE * WSCALE
NR = 4           # residual-corrected taps (0..3)

_COMPILED = {}


def _build_program():
    import concourse.bass as bass
    import concourse.tile as tile
    from concourse import bacc, mybir

    f32 = mybir.dt.float32
    bf16 = mybir.dt.bfloat16
    fp8 = mybir.dt.float8e4
    f16 = mybir.dt.float16
    Alu = mybir.AluOpType
    Act = mybir.ActivationFunctionType
    DR = mybir.MatmulPerfMode.DoubleRow

    nc = bacc.Bacc(
        "TRN2",
        target_bir_lowering=False,
        debug=False,
        num_devices=N_CORES,
    )

    x_d = nc.dram_tensor("x", [T, B_LOC, C, L], f32, kind="ExternalInput").ap()
    # [ci, k, co_t, ci_t, co] single e4m3 at scale 2^13
    ws_d = nc.dram_tensor("ws", [128, K, CT, 2, 128], fp8, kind="ExternalInput").ap()
    # residuals for taps 0..NR-1, same layout/scale
    wr_d = nc.dram_tensor("wr", [128, NR, CT, 2, 128], fp8, kind="ExternalInput").ap()
    # [co, field, m(rep), co_t]; fields: b' (=2^13 b), gamma, beta
    chan_d = nc.dram_tensor("chan", [128, 3, 2, CT], f32, kind="ExternalInput").ap()
    onesg_d = nc.dram_tensor("onesg", [128, 4], f32, kind="ExternalInput").ap()
    onesb4_d = nc.dram_tensor("onesb4", [4, 128], f32, kind="ExternalInput").ap()
    y_d = nc.dram_tensor("y", [T, B_LOC, C, L], f16, kind="ExternalOutput").ap()

    with tile.TileContext(nc) as tc:
        with (
            tc.tile_pool(name="singles", bufs=1) as singles,
            tc.tile_pool(name="xp", bufs=10) as xp,
            tc.tile_pool(name="sp", bufs=6) as sp,
            tc.tile_pool(name="ysb", bufs=8) as ysb,
            tc.tile_pool(name="ysb32", bufs=6) as ysb32,
            tc.tile_pool(name="smallsb", bufs=10) as smallsb,
            tc.tile_pool(name="ypsum", bufs=6, space="PSUM") as ypsum,
            tc.tile_pool(name="spsum", bufs=2, space="PSUM") as spsum,
        ):
            # PE p-state warmup: dummy matmuls on a memset tile keep PE busy
            # from ~0 so real convs start at full clock (off critical path --
            # they overlap the initial x DMA + LIF).
            warm_sb = singles.tile([128, 64], bf16)
            nc.vector.memset(warm_sb[:], 0.25)
            warm_ps = spsum.tile([128, 32], f32, name="small_ps")
            for _ in range(170):
                nc.tensor.matmul(
                    warm_ps[0:64, 0:8], warm_sb[:, 0:64], warm_sb[:, 0:8],
                    start=True, stop=True, skip_group_check=True,
                )
            first_small_ps = warm_ps
            # first input tiles first (LIF -> conv chain gates startup)
            early_x = {}
            for b in range(2):
                xt = xp.tile([128, 2, L], f32)
                nc.sync.dma_start(
                    out=xt[:], in_=x_d[0, b].rearrange("(i p) l -> p i l", p=128)
                )
                early_x[(0, b)] = xt
            ws = singles.tile([128, K, CT, 2, 128], fp8)
            nc.sync.dma_start(out=ws[:], in_=ws_d[:])
            wr = singles.tile([128, NR, CT, 2, 128], fp8)
            nc.sync.dma_start(out=wr[:], in_=wr_d[:])
            onesg = singles.tile([128, 4], f32)
            nc.sync.dma_start(out=onesg[:], in_=onesg_d[:])
            onesb4 = singles.tile([4, 128], f32)
            nc.sync.dma_start(out=onesb4[:], in_=onesb4_d[:])
            chan = singles.tile([128, 3, 2, CT], f32)
            nc.sync.dma_start(out=chan[:], in_=chan_d[:])
            eps_t = singles.tile([128, 1], f32)
            nc.vector.memset(eps_t[:], EPS_S)
            # pre-load the activation table (Sqrt selects sqrt_and_others,
            # which also holds Copy/Identity/Square) off the critical path
            eps_s = singles.tile([128, 1], f32)
            nc.scalar.activation(out=eps_s[0:1], in_=eps_t[0:1], func=Act.Sqrt)

            # persistent LIF membrane state (u = 2v scaling) per local batch;
            # first written at t=0 (no memset needed)
            m_tiles = []
            for b in range(B_LOC):
                mt = singles.tile([128, 2, L], f32, tag=f"m{b}")
                m_tiles.append(mt)

            # tap -> (rhs_lo, rhs_hi, out_lo, out_hi) column ranges
            tap_slices = []
            for k in range(K):
                d = k - 2
                if d >= 0:
                    tap_slices.append((d, L, 0, L - d))
                else:
                    tap_slices.append((0, L + d, -d, L))

            # matmul order: tap 2 single first (full-width start=True zeroes
            # the whole bank), then remaining singles, then residuals
            mm_list = [("s", 2), ("s", 0), ("s", 1), ("s", 3), ("s", 4)]
            mm_list += [("r", j) for j in range(NR)]
            n_mm = len(mm_list)

            def tail_front(pend):
                """Pair-tail stage 1: a/z per-channel stats (DVE) and the
                group-sum f32 matmul (PE)."""
                tb_pair, small_ps, stats, az, yps = pend
                gm = stats.shape[1]
                # a = mean + b'
                nc.vector.tensor_add(
                    out=az[:, :, :, 0], in0=stats[:, :, :, 0], in1=chan[:, 0, 0:gm]
                )
                # z = a^2 + var
                tmp = smallsb.tile([128, gm, CT], f32)
                nc.vector.tensor_mul(out=tmp[:], in0=az[:, :, :, 0], in1=az[:, :, :, 0])
                nc.vector.tensor_add(out=az[:, :, :, 1], in0=tmp[:], in1=stats[:, :, :, 1])
                # group sums: [4, gm*CT*2] f32 matmul
                nc.tensor.matmul(
                    small_ps[0:4, 0 : gm * 4], onesg[:], az[:],
                    start=True, stop=True,
                )

            def tail_mid(pend):
                """Pair-tail stage 2: mu/kappa chain (DVE+Act), broadcast
                matmul (PE), A/B coefficients (DVE)."""
                tb_pair, small_ps, stats, az, yps = pend
                gm = stats.shape[1]
                gsum = small_ps[0:4, 0 : gm * 4].rearrange(
                    "p (m c s) -> p m c s", m=gm, c=CT
                )
                mk = smallsb.tile([4, gm, CT, 2], f32)  # (mu, kappa)
                m2 = smallsb.tile([4, gm, CT], f32)
                vr = smallsb.tile([4, gm, CT], f32)
                mu_v = mk[0:4, :, :, 0]
                nc.vector.tensor_scalar(
                    out=mu_v, in0=gsum[:, :, :, 0], scalar1=1.0 / GPC,
                    scalar2=None, op0=Alu.mult,
                )
                nc.vector.tensor_mul(out=m2[:], in0=mu_v, in1=mu_v)
                nc.vector.scalar_tensor_tensor(
                    out=vr[:], in0=gsum[:, :, :, 1], scalar=1.0 / GPC, in1=m2[:],
                    op0=Alu.mult, op1=Alu.subtract,
                )
                nc.scalar.activation(
                    out=vr[:], in_=vr[:], func=Act.Sqrt, bias=eps_t[0:4],
                )
                nc.vector.reciprocal(out=mk[0:4, :, :, 1], in_=vr[:])
                # broadcast groups -> channels: [128, gm*CT*2] f32 matmul
                nc.tensor.matmul(
                    small_ps[:, 16 : 16 + gm * 4], onesb4[:], mk[:],
                    start=True, stop=True,
                )
                bcv = small_ps[:, 16 : 16 + gm * 4].rearrange(
                    "p (m c s) -> p m c s", m=gm, c=CT
                )
                # A = kappa * gamma ; B = (b' - mu) * A + beta
                ab = smallsb.tile([128, gm, CT, 2], f32)
                tmp = smallsb.tile([128, gm, CT], f32)
                nc.vector.tensor_mul(
                    out=ab[:, :, :, 0], in0=bcv[:, :, :, 1], in1=chan[:, 1, 0:gm]
                )
                nc.vector.tensor_sub(
                    out=tmp[:], in0=chan[:, 0, 0:gm], in1=bcv[:, :, :, 0]
                )
                nc.vector.tensor_mul(out=tmp[:], in0=tmp[:], in1=ab[:, :, :, 0])
                nc.vector.tensor_add(
                    out=ab[:, :, :, 1], in0=tmp[:], in1=chan[:, 2, 0:gm]
                )
                return ab

            def tail_store(pend, ab):
                """Pair-tail stage 3: out = A*Y + B on ScalarE (f32 SBUF ->
                fp16 SBUF) and DMA out."""
                tb_pair, small_ps, stats, az, yps = pend
                for mi in range(len(tb_pair)):
                    t, b = tb_pair[mi]
                    for ct in range(CT):
                        y_sb = ysb.tile([128, L], f16)
                        nc.scalar.activation(
                            out=y_sb[:], in_=yps[mi][ct][:], func=Act.Identity,
                            scale=ab[:, mi, ct, 0:1], bias=ab[:, mi, ct, 1:2],
                        )
                        nc.sync.dma_start(
                            out=y_d[t, b].rearrange("(i p) l -> p i l", p=128)[:, ct, :],
                            in_=y_sb[:],
                        )

            groups = [(2 * i, 2 * i + 1) for i in range(16)]
            gof = {}
            for g in groups:
                for j, s_ in enumerate(g):
                    gof[s_] = (g, j)
            pending = None
            cur = None
            for t in range(T):
                for b in range(B_LOC):
                    idx = t * B_LOC + b
                    grp, mi = gof[idx]
                    gsz = len(grp)
                    if mi == 0:
                        if idx == 0:
                            small_ps = first_small_ps
                        else:
                            small_ps = spsum.tile([128, 32], f32, name="small_ps")
                        stats = smallsb.tile([128, gsz, CT, 2], f32, name="stats")
                        az = smallsb.tile([128, gsz, CT, 2], f32, name="az")
                        cur = ([None] * gsz, small_ps, stats, az, [None] * gsz)
                    cur[0][mi] = (t, b)

                    xt = early_x.pop((t, b), None)
                    if xt is None:
                        xt = xp.tile([128, 2, L], f32)
                        nc.sync.dma_start(
                            out=xt[:],
                            in_=x_d[t, b].rearrange("(i p) l -> p i l", p=128),
                        )
                    mt = m_tiles[b]
                    st = sp.tile([128, 2, L], fp8)
                    if t == 0:
                        # m uninitialized: u = x exactly
                        nc.vector.tensor_scalar(
                            out=st[:], in0=xt[:], scalar1=1.0, scalar2=None,
                            op0=Alu.is_ge,
                        )
                        nc.vector.scalar_tensor_tensor(
                            out=mt[:], in0=xt[:], scalar=1.0, in1=xt[:],
                            op0=Alu.is_lt, op1=Alu.mult,
                        )
                    else:
                        # LIF step: u = 0.5*m + x ; s = (u>=1) ; m = (u<1)*u
                        nc.vector.scalar_tensor_tensor(
                            out=mt[:], in0=mt[:], scalar=0.5, in1=xt[:],
                            op0=Alu.mult, op1=Alu.add,
                        )
                        nc.vector.tensor_scalar(
                            out=st[:], in0=mt[:], scalar1=1.0, scalar2=None,
                            op0=Alu.is_ge,
                        )
                        if t < T - 1:
                            nc.vector.scalar_tensor_tensor(
                                out=mt[:], in0=mt[:], scalar=1.0, in1=mt[:],
                                op0=Alu.is_lt, op1=Alu.mult,
                            )

                    # conv + stats per co-tile; copy PSUM -> SBUF right away
                    # (ScalarE) so PSUM banks recycle without waiting on the
                    # GN tail chain
                    yps = []
                    for ct in range(CT):
                        yp = ypsum.tile([128, L], f32)
                        for i, (kind, k) in enumerate(mm_list):
                            rl, rh, ol, oh = tap_slices[k]
                            w_ap = ws[:, k, ct] if kind == "s" else wr[:, k, ct]
                            nc.tensor.matmul(
                                yp[:, ol:oh],
                                w_ap,
                                st[:, :, rl:rh],
                                start=(i == 0),
                                stop=(i == n_mm - 1),
                                perf_mode=DR,
                                skip_group_check=True,
                            )
                        bns = smallsb.tile([128, 6], f32)
                        nc.vector.bn_stats(out=bns[:], in_=yp[:])
                        nc.vector.bn_aggr(out=cur[2][:, mi, ct, :], in_=bns[:])
                        y32 = ysb32.tile([128, L], f32)
                        nc.scalar.activation(out=y32[:], in_=yp[:], func=Act.Copy)
                        yps.append(y32)
                    cur[4][mi] = yps

                    if mi == 0 and pending is not None:
                        ab = tail_mid(pending)
                        tail_store(pending, ab)
                        pending = None
                    if mi == gsz - 1:
                        pending = cur
                        tail_front(pending)
            ab = tail_mid(pending)
            tail_store(pending, ab)

    nc.compile()
    return nc


def _prep_host_inputs(x, conv_w, conv_b, gamma, beta):
    x = np.asarray(x, dtype=np.float32)
    conv_w = np.asarray(conv_w, dtype=np.float32)
    conv_b = np.asarray(conv_b, dtype=np.float32)
    gamma = np.asarray(gamma, dtype=np.float32)
    beta = np.asarray(beta, dtype=np.float32)

    def q8(a):
        return a.astype(ml_dtypes.float8_e4m3).astype(np.float32)

    # [ci_t, ci, co_t, co, k] at scale 2^13
    Wt = conv_w.transpose(1, 0, 2)                      # [ci_g, co_g, k]
    W6 = Wt.reshape(2, 128, CT, 128, K) * np.float32(WSCALE)
    w8 = q8(W6)
    r8 = q8(W6 - w8)
    # ws[ci, k, ct, ci_t, co]
    ws_host = np.ascontiguousarray(
        w8.transpose(1, 4, 2, 0, 3).astype(ml_dtypes.float8_e4m3)
    )
    # wr[ci, j(tap), ct, ci_t, co] for taps 0..NR-1
    wr_host = np.ascontiguousarray(
        r8[:, :, :, :, 0:NR].transpose(1, 4, 2, 0, 3).astype(ml_dtypes.float8_e4m3)
    )

    fields = np.stack([conv_b * np.float32(WSCALE), gamma, beta])  # [3, 256]
    chan1 = fields.reshape(3, CT, 128).transpose(2, 0, 1)          # [128, 3, ct]
    chan = np.ascontiguousarray(
        np.broadcast_to(chan1[:, :, None, :], (128, 3, 2, CT))
    )

    onesg = np.zeros((128, 4), np.float32)
    for ci in range(128):
        onesg[ci, ci // GPC] = 1.0
    onesb4 = np.zeros((4, 128), np.float32)
    for co in range(128):
        onesb4[co // GPC, co] = 1.0

    shards = []
    for i in range(N_CORES):
        shards.append(
            {
                "x": np.ascontiguousarray(x[:, i * B_LOC : (i + 1) * B_LOC]),
                "ws": ws_host,
                "wr": wr_host,
                "chan": chan,
                "onesg": onesg,
                "onesb4": onesb4,
            }
        )
    return shards


def kernel(x, conv_w, conv_b, gamma, beta, _trace=False):
    from concourse.bass_utils import run_bass_kernel_spmd

    if "nc" not in _COMPILED:
        _COMPILED["nc"] = _build_program()
    nc = _COMPILED["nc"]

    in_maps = _prep_host_inputs(x, conv_w, conv_b, gamma, beta)
    res = run_bass_kernel_spmd(
        nc, in_maps, list(range(N_CORES)), trace=_trace
    )
    out = np.concatenate([r["y"] for r in res.results], axis=1).astype(np.float32)
    _COMPILED["last_result"] = res
    return out
